# revision 21
# baseline (speedup 1.0000x reference)
"""Trainium2 Bass kernel for nn_KWinnersBoost (top-k masking with boosting).

Takes FULL inputs, returns FULL outputs. Row-parallel across 8 NeuronCores
(512 rows each), SPMD via run_bass_kernel_spmd.

v3.1 architecture (per core, 4 chunks of 128 partition-rows x 8192):
  Bisection (6 rounds) localizes the per-row top-K threshold inside a tuned
  bracket [1.95, 2.165] (t* of this regime lies in [1.956, 2.156]).
  Four fully independent per-chunk chains; each chain's bookkeeping lives
  ON ITS PROBING ENGINE so no cross-engine hop ever waits behind a big op:
    chunk 3 on DVE: is_gt+accum probe (exact counts), then b = (cnt<=K+.5),
      T' = T + R_r*(1-2b) as two tiny tensor_scalars (R_r = 0.215/2^(r+2),
      the radius recursion replaces LO/HI midpoint updates).
    chunks 0-2 on ACT: Sign+accum probe (sign-sum S), then sigma =
      Sign(S - (E-2K-1.5)) in {-1,+1}, b = (1+sigma)/2, T' = -R_r*sigma + T
      as three tiny activations.  T tiles ping-pong to avoid WAR stalls.
  The only off-engine state (HI and M-at-HI select-free updates, feeding
  extraction) runs on the otherwise idle Pool engine off the critical path.
  The ACT issue order is skewed (c0 every other slot) so chunk completions
  stagger and the DVE extraction of early chunks overlaps late probes.
  Extraction per chunk: f = (x <= HI)*x in column halves (chunks 3,0: DVE
  is_le mask (2x) + Pool tensor_tensor mult; chunks 1,2: fused DVE
  scalar_tensor_tensor for the shortest tail), max8 per half merged -> C8 =
  ranks m+1..m+8 below HI, t_final = C8[ceil(K-m)] via the one-hot band =
  exact (K+1)-th largest; out = (x > t_final) as int8 (2x DVE) -> DMA.
  Host decodes out.astype(f32) and boost_out = where(out, 0, c) exactly
  (c = relu(global max)*boost_percent in fp32 as the reference computes;
  the mask itself is invariant to the constant boost c).
  Safety: stat-only verification (idx in [-0.26, 7.26], boundary value-tie
  C8[j-1]==C8[j]) -> flags; any violation falls back to the exact host
  path. Sign ties and bracket misses all land in these flags.
"""

import os
import sys

if "/opt/trn_rl_repo" not in sys.path:
    sys.path.insert(0, "/opt/trn_rl_repo")

import numpy as np

import concourse.bacc as bacc
import concourse.bass as bass
import concourse.tile as tile
from concourse import mybir
from concourse.bass_utils import run_bass_kernel_spmd

F32 = mybir.dt.float32
I8 = mybir.dt.int8
I32 = mybir.dt.int32

B, E = 4096, 8192
N_CORES = 8
ROWS = B // N_CORES          # 512
P = 128
NCH = ROWS // P              # 4 chunks
K = 164
N_IT = 6                     # bisection rounds
LO0, HI0 = 1.95, 2.165       # tuned initial bracket (t* in [1.956, 2.156])
T0 = (LO0 + HI0) / 2.0
H = E // 2
SGN_SIG = float(E - 2 * K) - 1.5   # sigma bias: S - 7862.5, never 0
AluOp = mybir.AluOpType
Sign = mybir.ActivationFunctionType.Sign
Identity = mybir.ActivationFunctionType.Identity
AxX = mybir.AxisListType.X

DVE_CHUNK = 3
STT_CHUNKS = (0, 1, 2, 3)    # fused-stt f on DVE (keeps Pool queue tiny)
WA = 4096                    # ACT columns for split probes (DVE takes rest)
SPLIT_PROBES = set()


def _build_body(tc, x_d, out_d, flags_d, ctx):
    nc = tc.nc

    xpool = ctx.enter_context(tc.tile_pool(name="xpool", bufs=1))
    spool = ctx.enter_context(tc.tile_pool(name="spool", bufs=1))
    fpool = ctx.enter_context(tc.tile_pool(name="fpool", bufs=2))
    mpool = ctx.enter_context(tc.tile_pool(name="mpool", bufs=2))
    opool = ctx.enter_context(tc.tile_pool(name="opool", bufs=2))
    st = ctx.enter_context(tc.tile_pool(name="st", bufs=1))
    dram = ctx.enter_context(tc.tile_pool(name="dram", bufs=1, space="DRAM"))

    x_t = [xpool.tile([P, E], F32, tag=f"x{c}", name=f"x{c}") for c in range(NCH)]
    junkA = spool.tile([P, E], I8, tag="junkA", name="junkA")
    junkV = spool.tile([P, E], I8, tag="junkV", name="junkV")

    def stat(tag, w=1):
        return st.tile([P, w], F32, tag=tag, name=tag)

    # per-chunk chain state
    Tp = [[stat(f"T{c}_{i}") for i in range(2)] for c in range(NCH)]
    HI = [stat(f"HI{c}") for c in range(NCH)]
    MR = [stat(f"MR{c}") for c in range(NCH)]
    RAW = [[stat(f"RAW{c}_{i}") for i in range(2)] for c in range(NCH)]
    SCB = [stat(f"SCB{c}") for c in range(NCH)]
    SG = [[stat(f"SG{c}_{i}") for i in range(2)] for c in range(NCH)]
    Bb = [[stat(f"B{c}_{i}") for i in range(2)] for c in range(NCH)]
    U_ = [stat(f"U{c}") for c in range(NCH)]
    U2 = [stat(f"U2{c}") for c in range(NCH)]
    CD = [[stat(f"CD{c}_{i}") for i in range(2)] for c in range(NCH)]
    SD = [[stat(f"SD{c}_{i}") for i in range(2)] for c in range(NCH)]
    SA = [[stat(f"SA{c}_{i}") for i in range(2)] for c in range(NCH)]

    IDXU = [stat(f"IDXU{c}") for c in range(NCH)]
    IDXC = [stat(f"IDXC{c}") for c in range(NCH)]
    TF = [stat(f"TF{c}") for c in range(NCH)]
    TM1 = [stat(f"TM1{c}") for c in range(NCH)]
    V1 = [stat(f"V1{c}") for c in range(NCH)]
    V2 = [stat(f"V2{c}") for c in range(NCH)]
    DQ = [stat(f"DQ{c}") for c in range(NCH)]
    C16 = [st.tile([P, 16], F32, tag=f"C16_{c}", name=f"C16_{c}") for c in range(NCH)]
    C8 = [st.tile([P, 8], F32, tag=f"C8_{c}", name=f"C8_{c}") for c in range(NCH)]
    OHA = [st.tile([P, 8], F32, tag=f"OHA{c}", name=f"OHA{c}") for c in range(NCH)]
    OHB = [st.tile([P, 8], F32, tag=f"OHB{c}", name=f"OHB{c}") for c in range(NCH)]
    OHD = [st.tile([P, 8], F32, tag=f"OHD{c}", name=f"OHD{c}") for c in range(NCH)]
    BAD = stat("BAD")
    SGB = stat("SGB")            # ACT sigma bias const: -(E-2K-1.5)
    HT = stat("HT")              # 0.5
    IOT8I = st.tile([P, 8], I32, tag="IOT8I", name="IOT8I")
    IOT8F = st.tile([P, 8], F32, tag="IOT8F", name="IOT8F")
    R3o = st.tile([1, P], F32, tag="R3o", name="R3o")
    FLG = st.tile([1, 2], F32, tag="FLG", name="FLG")
    tr3_d = dram.tile([1, P], F32, tag="tr3", name="tr3")

    # ---------------- consts ---------------------------------------------
    nc.vector.memset(BAD, 0.0)
    nc.vector.memset(SGB, -SGN_SIG)
    nc.vector.memset(HT, 0.5)
    for c in range(NCH):
        nc.vector.memset(Tp[c][0], T0)
        nc.vector.memset(HI[c], HI0)
        nc.vector.memset(MR[c], -99999.0)
    nc.gpsimd.iota(IOT8I, pattern=[[1, 8]], base=0, channel_multiplier=0)
    nc.vector.tensor_copy(IOT8F, IOT8I)

    # ---------------- input DMA (c3+c0 halves first for fast start) ------
    nc.sync.dma_start(out=x_t[0][:, :H], in_=x_d[0:P, :H])
    nc.sync.dma_start(out=x_t[3][:, :H], in_=x_d[3 * P : 4 * P, :H])
    nc.sync.dma_start(out=x_t[0][:, H:], in_=x_d[0:P, H:])
    nc.sync.dma_start(out=x_t[3][:, H:], in_=x_d[3 * P : 4 * P, H:])
    nc.sync.dma_start(out=x_t[1], in_=x_d[P : 2 * P, :])
    nc.sync.dma_start(out=x_t[2], in_=x_d[2 * P : 3 * P, :])

    def round_(c, r):
        Tcur = Tp[c][r % 2]
        Tnxt = Tp[c][(r + 1) % 2]
        Rp = 0.215 / (2 ** (r + 2))
        Rcur = RAW[c][r % 2]
        SGc = SG[c][r % 2]
        Bbc = Bb[c][r % 2]
        split = r == 0 and c in (0, 3)
        segs = [(0, H), (H, E)] if split else [(0, E)]
        accs = [Rcur, SCB[c]] if split else [Rcur]
        on_dve = c == DVE_CHUNK or (c == 0 and r >= 3)
        if c == 0 and r >= 3:
            # c0's late rounds ride DVE (frees 3 ACT slots); counts are
            # converted to sign-sum units u = E - 2*cnt so MR stays uniform
            nc.vector.tensor_scalar(
                out=junkV, in0=x_t[c], scalar1=Tcur[:, 0:1], scalar2=0.0,
                op0=AluOp.is_gt, op1=AluOp.add, accum_out=CD[c][r % 2],
            )
            nc.vector.tensor_scalar(
                out=Rcur, in0=CD[c][r % 2], scalar1=-2.0, scalar2=float(E),
                op0=AluOp.mult, op1=AluOp.add,
            )
            # b = (u >= E-2K-1) <=> cnt <= K+0.5
            nc.vector.tensor_scalar(
                out=Bbc, in0=Rcur, scalar1=SGN_SIG, scalar2=None,
                op0=AluOp.is_ge,
            )
            if r < N_IT - 1:
                nc.vector.tensor_scalar(
                    out=SGc, in0=Bbc, scalar1=-2.0 * Rp, scalar2=Rp,
                    op0=AluOp.mult, op1=AluOp.add,
                )
                nc.vector.tensor_tensor(
                    out=Tnxt, in0=Tcur, in1=SGc, op=AluOp.add
                )
        elif c == DVE_CHUNK:
            for (c0_, c1_), a_ in zip(segs, accs):
                nc.vector.tensor_scalar(
                    out=junkV[:, : c1_ - c0_], in0=x_t[c][:, c0_:c1_],
                    scalar1=Tcur[:, 0:1], scalar2=0.0,
                    op0=AluOp.is_gt, op1=AluOp.add, accum_out=a_,
                )
            if split:
                nc.vector.tensor_tensor(
                    out=Rcur, in0=Rcur, in1=SCB[c], op=AluOp.add
                )
            # b = (cnt <= K+0.5); T' = T + R*(1-2b)   (all on DVE)
            nc.vector.tensor_scalar(
                out=Bbc, in0=Rcur, scalar1=float(K) + 0.5, scalar2=None,
                op0=AluOp.is_le,
            )
            if r < N_IT - 1:
                nc.vector.tensor_scalar(
                    out=SGc, in0=Bbc, scalar1=-2.0 * Rp, scalar2=Rp,
                    op0=AluOp.mult, op1=AluOp.add,
                )
                nc.vector.tensor_tensor(
                    out=Tnxt, in0=Tcur, in1=SGc, op=AluOp.add
                )
        else:
            if (c, r) in SPLIT_PROBES:
                # DVE takes cols [WA:E] (exact count), ACT cols [0:WA]
                # (sign-sum); u = S_act + (E-WA) - 2*C_dve is the exact
                # full-row sign-sum equivalent (DVE part tie-free).
                CDc, SDc, SAc = CD[c][r % 2], SD[c][r % 2], SA[c][r % 2]
                nc.vector.tensor_scalar(
                    out=junkV[:, : E - WA], in0=x_t[c][:, WA:],
                    scalar1=Tcur[:, 0:1], scalar2=0.0,
                    op0=AluOp.is_gt, op1=AluOp.add, accum_out=CDc,
                )
                nc.vector.tensor_scalar(
                    out=SDc, in0=CDc, scalar1=-2.0, scalar2=float(E - WA),
                    op0=AluOp.mult, op1=AluOp.add,
                )
                nc.scalar.activation(
                    out=junkA[:, :WA], in_=x_t[c][:, :WA],
                    func=Sign, bias=Tcur[:, 0:1], scale=-1.0, accum_out=SAc,
                )
                nc.scalar.activation(
                    out=Rcur, in_=SDc, func=Identity,
                    bias=SAc[:, 0:1], scale=1.0,
                )
            else:
                for (c0_, c1_), a_ in zip(segs, accs):
                    nc.scalar.activation(
                        out=junkA[:, : c1_ - c0_], in_=x_t[c][:, c0_:c1_],
                        func=Sign, bias=Tcur[:, 0:1], scale=-1.0, accum_out=a_,
                    )
                if split:
                    nc.scalar.activation(
                        out=Rcur, in_=SCB[c], func=Identity,
                        bias=Rcur[:, 0:1], scale=1.0,
                    )
            # sigma = Sign(S - 7862.5); T' = -R*sigma + T
            nc.scalar.activation(
                out=SGc, in_=Rcur, func=Sign, bias=SGB[:, 0:1], scale=1.0
            )
            if r < N_IT - 1:
                nc.scalar.activation(
                    out=Tnxt, in_=SGc, func=Identity,
                    bias=Tcur[:, 0:1], scale=-Rp,
                )
        # Pool, off the critical path: M_at_HI and HI select-free updates
        if on_dve:
            nc.gpsimd.tensor_tensor(out=U_[c], in0=Rcur, in1=MR[c], op=AluOp.subtract)
            nc.gpsimd.tensor_tensor(out=U_[c], in0=U_[c], in1=Bbc, op=AluOp.mult)
            nc.gpsimd.tensor_tensor(out=MR[c], in0=MR[c], in1=U_[c], op=AluOp.add)
            nc.gpsimd.tensor_tensor(out=U2[c], in0=Tcur, in1=HI[c], op=AluOp.subtract)
            nc.gpsimd.tensor_tensor(out=U2[c], in0=U2[c], in1=Bbc, op=AluOp.mult)
            nc.gpsimd.tensor_tensor(out=HI[c], in0=HI[c], in1=U2[c], op=AluOp.add)
        else:
            # x_new = x + (v-x)*(1+sigma)/2 via d, d*sg, sum, half, add
            nc.gpsimd.tensor_tensor(out=U_[c], in0=Rcur, in1=MR[c], op=AluOp.subtract)
            nc.gpsimd.tensor_tensor(out=U2[c], in0=U_[c], in1=SGc, op=AluOp.mult)
            nc.gpsimd.tensor_tensor(out=U_[c], in0=U_[c], in1=U2[c], op=AluOp.add)
            nc.gpsimd.tensor_tensor(out=U_[c], in0=U_[c], in1=HT, op=AluOp.mult)
            nc.gpsimd.tensor_tensor(out=MR[c], in0=MR[c], in1=U_[c], op=AluOp.add)
            nc.gpsimd.tensor_tensor(out=U_[c], in0=Tcur, in1=HI[c], op=AluOp.subtract)
            nc.gpsimd.tensor_tensor(out=U2[c], in0=U_[c], in1=SGc, op=AluOp.mult)
            nc.gpsimd.tensor_tensor(out=U_[c], in0=U_[c], in1=U2[c], op=AluOp.add)
            nc.gpsimd.tensor_tensor(out=U_[c], in0=U_[c], in1=HT, op=AluOp.mult)
            nc.gpsimd.tensor_tensor(out=HI[c], in0=HI[c], in1=U_[c], op=AluOp.add)

    def extract(c):
        r0 = c * P
        hic = HI[c][:, 0:1]
        # idx = K - m from raw units
        if c == DVE_CHUNK:
            nc.vector.tensor_scalar(
                out=IDXU[c], in0=MR[c], scalar1=-1.0, scalar2=float(K),
                op0=AluOp.mult, op1=AluOp.add,
            )
        else:
            nc.vector.tensor_scalar(
                out=IDXU[c], in0=MR[c], scalar1=0.5, scalar2=float(K) - E / 2.0,
                op0=AluOp.mult, op1=AluOp.add,
            )
        nc.vector.tensor_scalar(
            out=IDXC[c], in0=IDXU[c], scalar1=0.0, scalar2=7.0,
            op0=AluOp.max, op1=AluOp.min,
        )
        for h in range(2):
            c0_, c1_ = (0, H) if h == 0 else (H, E)
            f = fpool.tile([P, H], F32, tag="f", name=f"f{c}_{h}")
            if c in STT_CHUNKS:
                nc.vector.scalar_tensor_tensor(
                    out=f, in0=x_t[c][:, c0_:c1_], scalar=hic,
                    in1=x_t[c][:, c0_:c1_], op0=AluOp.is_le, op1=AluOp.mult,
                )
            else:
                mk = mpool.tile([P, H], I8, tag="mk", name=f"mk{c}_{h}")
                nc.vector.tensor_scalar(
                    out=mk, in0=x_t[c][:, c0_:c1_], scalar1=hic,
                    scalar2=None, op0=AluOp.is_le,
                )
                nc.gpsimd.tensor_tensor(
                    out=f, in0=x_t[c][:, c0_:c1_], in1=mk, op=AluOp.mult
                )
            nc.vector.max(out=C16[c][:, 8 * h : 8 * h + 8], in_=f)
        nc.vector.max(out=C8[c], in_=C16[c])
        # one-hot band around ceil(idx)
        nc.vector.tensor_scalar(
            out=OHA[c], in0=IOT8F, scalar1=IDXC[c][:, 0:1], scalar2=None,
            op0=AluOp.subtract,
        )
        nc.vector.tensor_scalar(
            out=OHB[c], in0=OHA[c], scalar1=-0.26, scalar2=None, op0=AluOp.is_ge
        )
        nc.vector.tensor_scalar(
            out=OHA[c], in0=OHA[c], scalar1=0.76, scalar2=None, op0=AluOp.is_le
        )
        nc.vector.tensor_tensor(out=OHA[c], in0=OHA[c], in1=OHB[c], op=AluOp.mult)
        nc.vector.tensor_tensor(out=OHD[c], in0=OHA[c], in1=C8[c], op=AluOp.mult)
        nc.vector.reduce_sum(out=TF[c], in_=OHD[c], axis=AxX)
        nc.vector.tensor_tensor(
            out=OHD[c][:, 0:7], in0=OHA[c][:, 1:8], in1=C8[c][:, 0:7],
            op=AluOp.mult,
        )
        nc.vector.reduce_sum(out=TM1[c], in_=OHD[c][:, 0:7], axis=AxX)
        # out = (x > t_final) as int8, then DMA
        om = opool.tile([P, E], I8, tag="om", name=f"om{c}")
        nc.vector.tensor_scalar(
            out=om, in0=x_t[c], scalar1=TF[c][:, 0:1], scalar2=None,
            op0=AluOp.is_gt,
        )
        nc.sync.dma_start(out=out_d[r0 : r0 + P, :], in_=om)
        # flags: idx out of band range, or boundary value-tie
        nc.vector.tensor_scalar(
            out=V1[c], in0=IDXU[c], scalar1=-0.26, scalar2=None, op0=AluOp.is_lt
        )
        nc.vector.tensor_scalar(
            out=V2[c], in0=IDXU[c], scalar1=7.26, scalar2=None, op0=AluOp.is_gt
        )
        nc.vector.tensor_tensor(out=DQ[c], in0=TF[c], in1=TM1[c], op=AluOp.subtract)
        nc.vector.tensor_scalar(
            out=DQ[c], in0=DQ[c], scalar1=0.0, scalar2=None, op0=AluOp.is_equal
        )
        nc.vector.tensor_scalar(
            out=TM1[c], in0=IDXU[c], scalar1=0.74, scalar2=None, op0=AluOp.is_ge
        )
        nc.vector.tensor_tensor(out=DQ[c], in0=DQ[c], in1=TM1[c], op=AluOp.mult)
        nc.vector.tensor_tensor(out=BAD, in0=BAD, in1=V1[c], op=AluOp.add)
        nc.vector.tensor_tensor(out=BAD, in0=BAD, in1=V2[c], op=AluOp.add)
        nc.vector.tensor_tensor(out=BAD, in0=BAD, in1=DQ[c], op=AluOp.add)

    # ------------- staggered issue: c3 DVE-paced, c0 ACT-priority ---------
    ORDER = [
        (3, 0), (0, 0), (1, 0),
        (3, 1), (0, 1), (2, 0),
        (3, 2), (0, 2), (1, 1),
        (2, 1), (3, 3), (1, 2),
        (2, 2), (3, 4), (1, 3),
        (3, 5), (0, 3), (0, 4), (0, 5), "E3", "E0",
        (2, 3), (1, 4), (2, 4), (1, 5), "E1", (2, 5), "E2",
    ]
    for tok in ORDER:
        if isinstance(tok, str):
            extract(int(tok[1]))
        else:
            round_(*tok)

    # ---------------- flags: raw per-partition sums, host adds -----------
    nc.sync.dma_start(out=flags_d[:, :], in_=BAD)


_NC_CACHE = None


def _build():
    global _NC_CACHE
    if _NC_CACHE is not None:
        return _NC_CACHE
    nc = bacc.Bacc(
        "TRN2", target_bir_lowering=False, debug=False, num_devices=N_CORES
    )
    x_d = nc.dram_tensor("tensor", [ROWS, E], F32, kind="ExternalInput").ap()
    out_d = nc.dram_tensor("out", [ROWS, E], I8, kind="ExternalOutput").ap()
    flags_d = nc.dram_tensor("flags", [P, 1], F32, kind="ExternalOutput").ap()
    from contextlib import ExitStack

    with tile.TileContext(nc) as tc, ExitStack() as ctx:
        _build_body(tc, x_d, out_d, flags_d, ctx)
    nc.compile()
    _NC_CACHE = nc
    return nc


_LAST_RESULTS = None


def kernel(tensor, boost_tensor, boost_percent):
    global _LAST_RESULTS
    tensor = np.ascontiguousarray(np.asarray(tensor, dtype=np.float32))
    boost_tensor = np.asarray(boost_tensor, dtype=np.float32)
    bp = np.float32(np.asarray(boost_percent, dtype=np.float32).reshape(-1)[0])

    # device path assumes boost_tensor == 0 (this module's forward contract);
    # exotic nonzero boosts take the exact host path
    if boost_tensor.any():
        return _host_reference(tensor, boost_tensor, float(bp))

    # c = relu(global max) * boost_percent, in fp32 exactly as the reference
    gmax = np.float32(max(np.float32(0.0), tensor.max()))
    cval = np.float32(gmax * bp)

    nc = _build()
    in_maps = []
    for c in range(N_CORES):
        sl = slice(c * ROWS, (c + 1) * ROWS)
        in_maps.append({"tensor": tensor[sl]})
    trace = bool(int(os.environ.get("KW_TRACE", "0")))
    res = run_bass_kernel_spmd(
        nc, in_maps, core_ids=list(range(N_CORES)), trace=trace
    )
    _LAST_RESULTS = res

    nbad = sum(float(np.asarray(r["flags"]).sum()) for r in res.results)
    if nbad > 0:
        return _host_reference(tensor, boost_tensor, float(bp))

    mask = np.concatenate(
        [np.asarray(r["out"]) for r in res.results], axis=0
    )
    out = mask.astype(np.float32)
    bo = np.where(mask != 0, np.float32(0.0), cval).astype(np.float32)
    return out, bo


def _host_reference(tensor, boost_tensor, bp):
    x = tensor.astype(np.float32)
    b = np.broadcast_to(boost_tensor.astype(np.float32), x.shape)
    max_val = max(0.0, float(x.max()))
    boost = (b + np.float32(max_val * bp)).astype(np.float32)
    boosted = (np.where(x > 0, x, np.float32(0)) + boost).astype(np.float32)
    kth = np.partition(boosted, E - K, axis=1)[:, E - K]
    mask = boosted > kth[:, None]
    need = K - mask.sum(1)
    tie = (boosted == kth[:, None]) & ~mask
    csum = np.cumsum(tie, axis=1)
    mask |= tie & (csum <= need[:, None])
    out = (mask & (x > 0)).astype(np.float32)
    if out.sum() == 0:
        out = mask.astype(np.float32)
    bo = np.where(mask, np.float32(0), boost).astype(np.float32)
    return out, bo


# revision 23
# speedup vs baseline: 1.1750x; 1.1750x over previous
"""Trainium2 Bass kernel for nn_KWinnersBoost (top-k masking with boosting).

Takes FULL inputs, returns FULL outputs. Row-parallel across 8 NeuronCores
(512 rows each), SPMD via run_bass_kernel_spmd.

v3.1 architecture (per core, 4 chunks of 128 partition-rows x 8192):
  Bisection (6 rounds) localizes the per-row top-K threshold inside a tuned
  bracket [1.95, 2.165] (t* of this regime lies in [1.956, 2.156]).
  Four fully independent per-chunk chains; each chain's bookkeeping lives
  ON ITS PROBING ENGINE so no cross-engine hop ever waits behind a big op:
    chunk 3 on DVE: is_gt+accum probe (exact counts), then b = (cnt<=K+.5),
      T' = T + R_r*(1-2b) as two tiny tensor_scalars (R_r = 0.215/2^(r+2),
      the radius recursion replaces LO/HI midpoint updates).
    chunks 0-2 on ACT: Sign+accum probe (sign-sum S), then sigma =
      Sign(S - (E-2K-1.5)) in {-1,+1}, b = (1+sigma)/2, T' = -R_r*sigma + T
      as three tiny activations.  T tiles ping-pong to avoid WAR stalls.
  The only off-engine state (HI and M-at-HI select-free updates, feeding
  extraction) runs on the otherwise idle Pool engine off the critical path.
  The ACT issue order is skewed (c0 every other slot) so chunk completions
  stagger and the DVE extraction of early chunks overlaps late probes.
  Extraction per chunk: f = (x <= HI)*x in column halves (chunks 3,0: DVE
  is_le mask (2x) + Pool tensor_tensor mult; chunks 1,2: fused DVE
  scalar_tensor_tensor for the shortest tail), max8 per half merged -> C8 =
  ranks m+1..m+8 below HI, t_final = C8[ceil(K-m)] via the one-hot band =
  exact (K+1)-th largest; out = (x > t_final) as int8 (2x DVE) -> DMA.
  Host decodes out.astype(f32) and boost_out = where(out, 0, c) exactly
  (c = relu(global max)*boost_percent in fp32 as the reference computes;
  the mask itself is invariant to the constant boost c).
  Safety: stat-only verification (idx in [-0.26, 7.26], boundary value-tie
  C8[j-1]==C8[j]) -> flags; any violation falls back to the exact host
  path. Sign ties and bracket misses all land in these flags.
"""

import os
import sys

if "/opt/trn_rl_repo" not in sys.path:
    sys.path.insert(0, "/opt/trn_rl_repo")

import numpy as np

import concourse.bacc as bacc
import concourse.bass as bass
import concourse.tile as tile
from concourse import mybir
from concourse.bass_utils import run_bass_kernel_spmd

F32 = mybir.dt.float32
I8 = mybir.dt.int8
I32 = mybir.dt.int32

B, E = 4096, 8192
N_CORES = 8
ROWS = B // N_CORES          # 512
P = 128
NCH = ROWS // P              # 4 chunks
K = 164
N_IT = 6                     # bisection rounds
LO0, HI0 = 1.95, 2.165       # tuned initial bracket (t* in [1.956, 2.156])
T0 = (LO0 + HI0) / 2.0
H = E // 2
SGN_SIG = float(E - 2 * K) - 1.5   # sigma bias: S - 7862.5, never 0
AluOp = mybir.AluOpType
Sign = mybir.ActivationFunctionType.Sign
Identity = mybir.ActivationFunctionType.Identity
AxX = mybir.AxisListType.X

DVE_CHUNK = 3
STT_CHUNKS = (0, 1, 2, 3)    # fused-stt f on DVE (keeps Pool queue tiny)
WA = 4096                    # ACT columns for split probes (DVE takes rest)
SPLIT_PROBES = set()


def _build_body(tc, x_d, out_d, flags_d, ctx):
    nc = tc.nc

    xpool = ctx.enter_context(tc.tile_pool(name="xpool", bufs=1))
    spool = ctx.enter_context(tc.tile_pool(name="spool", bufs=1))
    fpool = ctx.enter_context(tc.tile_pool(name="fpool", bufs=2))
    mpool = ctx.enter_context(tc.tile_pool(name="mpool", bufs=2))
    opool = ctx.enter_context(tc.tile_pool(name="opool", bufs=2))
    st = ctx.enter_context(tc.tile_pool(name="st", bufs=1))
    dram = ctx.enter_context(tc.tile_pool(name="dram", bufs=1, space="DRAM"))

    x_t = [xpool.tile([P, E], F32, tag=f"x{c}", name=f"x{c}") for c in range(NCH)]
    junkA = spool.tile([P, E], I8, tag="junkA", name="junkA")
    junkV = spool.tile([P, E], I8, tag="junkV", name="junkV")

    def stat(tag, w=1):
        return st.tile([P, w], F32, tag=tag, name=tag)

    # per-chunk chain state
    Tp = [[stat(f"T{c}_{i}") for i in range(2)] for c in range(NCH)]
    HI = [stat(f"HI{c}") for c in range(NCH)]
    MR = [stat(f"MR{c}") for c in range(NCH)]
    RAW = [[stat(f"RAW{c}_{i}") for i in range(2)] for c in range(NCH)]
    SCB = [stat(f"SCB{c}") for c in range(NCH)]
    SG = [[stat(f"SG{c}_{i}") for i in range(2)] for c in range(NCH)]
    Bb = [[stat(f"B{c}_{i}") for i in range(2)] for c in range(NCH)]
    U_ = [stat(f"U{c}") for c in range(NCH)]
    U2 = [stat(f"U2{c}") for c in range(NCH)]
    CD = [[stat(f"CD{c}_{i}") for i in range(2)] for c in range(NCH)]
    SD = [[stat(f"SD{c}_{i}") for i in range(2)] for c in range(NCH)]
    SA = [[stat(f"SA{c}_{i}") for i in range(2)] for c in range(NCH)]

    IDXU = [stat(f"IDXU{c}") for c in range(NCH)]
    IDXC = [stat(f"IDXC{c}") for c in range(NCH)]
    TF = [stat(f"TF{c}") for c in range(NCH)]
    TM1 = [stat(f"TM1{c}") for c in range(NCH)]
    V1 = [stat(f"V1{c}") for c in range(NCH)]
    V2 = [stat(f"V2{c}") for c in range(NCH)]
    DQ = [stat(f"DQ{c}") for c in range(NCH)]
    C16 = [st.tile([P, 16], F32, tag=f"C16_{c}", name=f"C16_{c}") for c in range(NCH)]
    C8 = [st.tile([P, 8], F32, tag=f"C8_{c}", name=f"C8_{c}") for c in range(NCH)]
    OHA = [st.tile([P, 8], F32, tag=f"OHA{c}", name=f"OHA{c}") for c in range(NCH)]
    OHB = [st.tile([P, 8], F32, tag=f"OHB{c}", name=f"OHB{c}") for c in range(NCH)]
    OHD = [st.tile([P, 8], F32, tag=f"OHD{c}", name=f"OHD{c}") for c in range(NCH)]
    BAD = stat("BAD")
    SGB = stat("SGB")            # ACT sigma bias const: -(E-2K-1.5)
    HT = stat("HT")              # 0.5
    IOT8I = st.tile([P, 8], I32, tag="IOT8I", name="IOT8I")
    IOT8F = st.tile([P, 8], F32, tag="IOT8F", name="IOT8F")
    R3o = st.tile([1, P], F32, tag="R3o", name="R3o")
    FLG = st.tile([1, 2], F32, tag="FLG", name="FLG")
    tr3_d = dram.tile([1, P], F32, tag="tr3", name="tr3")

    # ---------------- consts ---------------------------------------------
    nc.vector.memset(BAD, 0.0)
    nc.vector.memset(SGB, -SGN_SIG)
    nc.vector.memset(HT, 0.5)
    for c in range(NCH):
        nc.vector.memset(Tp[c][0], T0)
        nc.vector.memset(HI[c], HI0)
        nc.vector.memset(MR[c], -99999.0)
    nc.gpsimd.iota(IOT8I, pattern=[[1, 8]], base=0, channel_multiplier=0)
    nc.vector.tensor_copy(IOT8F, IOT8I)

    # ---------------- input DMA (c3+c0 halves first for fast start) ------
    nc.sync.dma_start(out=x_t[0][:, :H], in_=x_d[0:P, :H])
    nc.sync.dma_start(out=x_t[3][:, :H], in_=x_d[3 * P : 4 * P, :H])
    nc.sync.dma_start(out=x_t[0][:, H:], in_=x_d[0:P, H:])
    nc.sync.dma_start(out=x_t[3][:, H:], in_=x_d[3 * P : 4 * P, H:])
    nc.sync.dma_start(out=x_t[1], in_=x_d[P : 2 * P, :])
    nc.sync.dma_start(out=x_t[2], in_=x_d[2 * P : 3 * P, :])

    def round_(c, r):
        Tcur = Tp[c][r % 2]
        Tnxt = Tp[c][(r + 1) % 2]
        Rp = 0.215 / (2 ** (r + 2))
        Rcur = RAW[c][r % 2]
        SGc = SG[c][r % 2]
        Bbc = Bb[c][r % 2]
        split = r == 0 and c in (0, 3)
        segs = [(0, H), (H, E)] if split else [(0, E)]
        accs = [Rcur, SCB[c]] if split else [Rcur]
        on_dve = c == DVE_CHUNK or (c == 0 and r >= 3)
        if c == 0 and r >= 3:
            # c0's late rounds ride DVE (frees 3 ACT slots); counts are
            # converted to sign-sum units u = E - 2*cnt so MR stays uniform
            nc.vector.tensor_scalar(
                out=junkV, in0=x_t[c], scalar1=Tcur[:, 0:1], scalar2=0.0,
                op0=AluOp.is_gt, op1=AluOp.add, accum_out=CD[c][r % 2],
            )
            nc.vector.tensor_scalar(
                out=Rcur, in0=CD[c][r % 2], scalar1=-2.0, scalar2=float(E),
                op0=AluOp.mult, op1=AluOp.add,
            )
            # b = (u >= E-2K-1) <=> cnt <= K+0.5
            nc.vector.tensor_scalar(
                out=Bbc, in0=Rcur, scalar1=SGN_SIG, scalar2=None,
                op0=AluOp.is_ge,
            )
            if r < N_IT - 1:
                nc.vector.tensor_scalar(
                    out=SGc, in0=Bbc, scalar1=-2.0 * Rp, scalar2=Rp,
                    op0=AluOp.mult, op1=AluOp.add,
                )
                nc.vector.tensor_tensor(
                    out=Tnxt, in0=Tcur, in1=SGc, op=AluOp.add
                )
        elif c == DVE_CHUNK:
            for (c0_, c1_), a_ in zip(segs, accs):
                nc.vector.tensor_scalar(
                    out=junkV[:, : c1_ - c0_], in0=x_t[c][:, c0_:c1_],
                    scalar1=Tcur[:, 0:1], scalar2=0.0,
                    op0=AluOp.is_gt, op1=AluOp.add, accum_out=a_,
                )
            if split:
                nc.vector.tensor_tensor(
                    out=Rcur, in0=Rcur, in1=SCB[c], op=AluOp.add
                )
            # b = (cnt <= K+0.5); T' = T + R*(1-2b)   (all on DVE)
            nc.vector.tensor_scalar(
                out=Bbc, in0=Rcur, scalar1=float(K) + 0.5, scalar2=None,
                op0=AluOp.is_le,
            )
            if r < N_IT - 1:
                nc.vector.tensor_scalar(
                    out=SGc, in0=Bbc, scalar1=-2.0 * Rp, scalar2=Rp,
                    op0=AluOp.mult, op1=AluOp.add,
                )
                nc.vector.tensor_tensor(
                    out=Tnxt, in0=Tcur, in1=SGc, op=AluOp.add
                )
        else:
            if (c, r) in SPLIT_PROBES:
                # DVE takes cols [WA:E] (exact count), ACT cols [0:WA]
                # (sign-sum); u = S_act + (E-WA) - 2*C_dve is the exact
                # full-row sign-sum equivalent (DVE part tie-free).
                CDc, SDc, SAc = CD[c][r % 2], SD[c][r % 2], SA[c][r % 2]
                nc.vector.tensor_scalar(
                    out=junkV[:, : E - WA], in0=x_t[c][:, WA:],
                    scalar1=Tcur[:, 0:1], scalar2=0.0,
                    op0=AluOp.is_gt, op1=AluOp.add, accum_out=CDc,
                )
                nc.vector.tensor_scalar(
                    out=SDc, in0=CDc, scalar1=-2.0, scalar2=float(E - WA),
                    op0=AluOp.mult, op1=AluOp.add,
                )
                nc.scalar.activation(
                    out=junkA[:, :WA], in_=x_t[c][:, :WA],
                    func=Sign, bias=Tcur[:, 0:1], scale=-1.0, accum_out=SAc,
                )
                nc.scalar.activation(
                    out=Rcur, in_=SDc, func=Identity,
                    bias=SAc[:, 0:1], scale=1.0,
                )
            else:
                for (c0_, c1_), a_ in zip(segs, accs):
                    nc.scalar.activation(
                        out=junkA[:, : c1_ - c0_], in_=x_t[c][:, c0_:c1_],
                        func=Sign, bias=Tcur[:, 0:1], scale=-1.0, accum_out=a_,
                    )
                if split:
                    nc.scalar.activation(
                        out=Rcur, in_=SCB[c], func=Identity,
                        bias=Rcur[:, 0:1], scale=1.0,
                    )
            # sigma = Sign(S - 7862.5); T' = -R*sigma + T
            nc.scalar.activation(
                out=SGc, in_=Rcur, func=Sign, bias=SGB[:, 0:1], scale=1.0
            )
            if r < N_IT - 1:
                nc.scalar.activation(
                    out=Tnxt, in_=SGc, func=Identity,
                    bias=Tcur[:, 0:1], scale=-Rp,
                )
        # Pool, off the critical path: M_at_HI and HI select-free updates
        if on_dve:
            nc.gpsimd.tensor_tensor(out=U_[c], in0=Rcur, in1=MR[c], op=AluOp.subtract)
            nc.gpsimd.tensor_tensor(out=U_[c], in0=U_[c], in1=Bbc, op=AluOp.mult)
            nc.gpsimd.tensor_tensor(out=MR[c], in0=MR[c], in1=U_[c], op=AluOp.add)
            nc.gpsimd.tensor_tensor(out=U2[c], in0=Tcur, in1=HI[c], op=AluOp.subtract)
            nc.gpsimd.tensor_tensor(out=U2[c], in0=U2[c], in1=Bbc, op=AluOp.mult)
            nc.gpsimd.tensor_tensor(out=HI[c], in0=HI[c], in1=U2[c], op=AluOp.add)
        else:
            # x_new = x + (v-x)*(1+sigma)/2 via d, d*sg, sum, half, add
            nc.gpsimd.tensor_tensor(out=U_[c], in0=Rcur, in1=MR[c], op=AluOp.subtract)
            nc.gpsimd.tensor_tensor(out=U2[c], in0=U_[c], in1=SGc, op=AluOp.mult)
            nc.gpsimd.tensor_tensor(out=U_[c], in0=U_[c], in1=U2[c], op=AluOp.add)
            nc.gpsimd.tensor_tensor(out=U_[c], in0=U_[c], in1=HT, op=AluOp.mult)
            nc.gpsimd.tensor_tensor(out=MR[c], in0=MR[c], in1=U_[c], op=AluOp.add)
            nc.gpsimd.tensor_tensor(out=U_[c], in0=Tcur, in1=HI[c], op=AluOp.subtract)
            nc.gpsimd.tensor_tensor(out=U2[c], in0=U_[c], in1=SGc, op=AluOp.mult)
            nc.gpsimd.tensor_tensor(out=U_[c], in0=U_[c], in1=U2[c], op=AluOp.add)
            nc.gpsimd.tensor_tensor(out=U_[c], in0=U_[c], in1=HT, op=AluOp.mult)
            nc.gpsimd.tensor_tensor(out=HI[c], in0=HI[c], in1=U_[c], op=AluOp.add)

    def extract(c):
        r0 = c * P
        hic = HI[c][:, 0:1]
        # idx = K - m from raw units
        if c == DVE_CHUNK:
            nc.vector.tensor_scalar(
                out=IDXU[c], in0=MR[c], scalar1=-1.0, scalar2=float(K),
                op0=AluOp.mult, op1=AluOp.add,
            )
        else:
            nc.vector.tensor_scalar(
                out=IDXU[c], in0=MR[c], scalar1=0.5, scalar2=float(K) - E / 2.0,
                op0=AluOp.mult, op1=AluOp.add,
            )
        nc.vector.tensor_scalar(
            out=IDXC[c], in0=IDXU[c], scalar1=0.0, scalar2=7.0,
            op0=AluOp.max, op1=AluOp.min,
        )
        for h in range(2):
            c0_, c1_ = (0, H) if h == 0 else (H, E)
            f = fpool.tile([P, H], F32, tag="f", name=f"f{c}_{h}")
            if c in STT_CHUNKS:
                nc.vector.scalar_tensor_tensor(
                    out=f, in0=x_t[c][:, c0_:c1_], scalar=hic,
                    in1=x_t[c][:, c0_:c1_], op0=AluOp.is_le, op1=AluOp.mult,
                )
            else:
                mk = mpool.tile([P, H], I8, tag="mk", name=f"mk{c}_{h}")
                nc.vector.tensor_scalar(
                    out=mk, in0=x_t[c][:, c0_:c1_], scalar1=hic,
                    scalar2=None, op0=AluOp.is_le,
                )
                nc.gpsimd.tensor_tensor(
                    out=f, in0=x_t[c][:, c0_:c1_], in1=mk, op=AluOp.mult
                )
            nc.vector.max(out=C16[c][:, 8 * h : 8 * h + 8], in_=f)
        nc.vector.max(out=C8[c], in_=C16[c])
        # one-hot band around ceil(idx)
        nc.vector.tensor_scalar(
            out=OHA[c], in0=IOT8F, scalar1=IDXC[c][:, 0:1], scalar2=None,
            op0=AluOp.subtract,
        )
        nc.vector.tensor_scalar(
            out=OHB[c], in0=OHA[c], scalar1=-0.26, scalar2=None, op0=AluOp.is_ge
        )
        nc.vector.tensor_scalar(
            out=OHA[c], in0=OHA[c], scalar1=0.76, scalar2=None, op0=AluOp.is_le
        )
        nc.vector.tensor_tensor(out=OHA[c], in0=OHA[c], in1=OHB[c], op=AluOp.mult)
        nc.vector.tensor_tensor(out=OHD[c], in0=OHA[c], in1=C8[c], op=AluOp.mult)
        nc.vector.reduce_sum(out=TF[c], in_=OHD[c], axis=AxX)
        # out immediately after TF (critical tail); c1's out rides the
        # by-then-idle ACT as Sign(x - t_final) -- host decodes mask = out>0
        om = opool.tile([P, E], I8, tag="om", name=f"om{c}")
        if c == 1:
            nc.vector.tensor_scalar(
                out=SD[c][0], in0=TF[c], scalar1=-1.0, scalar2=None,
                op0=AluOp.mult,
            )
            nc.scalar.activation(
                out=om, in_=x_t[c], func=Sign, bias=SD[c][0][:, 0:1], scale=1.0,
            )
        else:
            nc.vector.tensor_scalar(
                out=om, in0=x_t[c], scalar1=TF[c][:, 0:1], scalar2=None,
                op0=AluOp.is_gt,
            )
        nc.sync.dma_start(out=out_d[r0 : r0 + P, :], in_=om)
        nc.vector.tensor_tensor(
            out=OHD[c][:, 0:7], in0=OHA[c][:, 1:8], in1=C8[c][:, 0:7],
            op=AluOp.mult,
        )
        nc.vector.reduce_sum(out=TM1[c], in_=OHD[c][:, 0:7], axis=AxX)
        # flags: idx out of band range, or boundary value-tie
        nc.vector.tensor_scalar(
            out=V1[c], in0=IDXU[c], scalar1=-0.26, scalar2=None, op0=AluOp.is_lt
        )
        nc.vector.tensor_scalar(
            out=V2[c], in0=IDXU[c], scalar1=7.26, scalar2=None, op0=AluOp.is_gt
        )
        nc.vector.tensor_tensor(out=DQ[c], in0=TF[c], in1=TM1[c], op=AluOp.subtract)
        nc.vector.tensor_scalar(
            out=DQ[c], in0=DQ[c], scalar1=0.0, scalar2=None, op0=AluOp.is_equal
        )
        nc.vector.tensor_scalar(
            out=TM1[c], in0=IDXU[c], scalar1=0.74, scalar2=None, op0=AluOp.is_ge
        )
        nc.vector.tensor_tensor(out=DQ[c], in0=DQ[c], in1=TM1[c], op=AluOp.mult)
        nc.vector.tensor_tensor(out=BAD, in0=BAD, in1=V1[c], op=AluOp.add)
        nc.vector.tensor_tensor(out=BAD, in0=BAD, in1=V2[c], op=AluOp.add)
        nc.vector.tensor_tensor(out=BAD, in0=BAD, in1=DQ[c], op=AluOp.add)

    # ------------- staggered issue: c3 DVE-paced, c0 ACT-priority ---------
    ORDER = [
        (3, 0), (0, 0), (1, 0),
        (3, 1), (0, 1), (2, 0),
        (3, 2), (0, 2), (1, 1),
        (2, 1), (3, 3), (1, 2),
        (2, 2), (3, 4), (1, 3),
        (3, 5), (0, 3), (0, 4), (0, 5), "E3", "E0",
        (2, 3), (1, 4), (2, 4), (1, 5), "E1", (2, 5), "E2",
    ]
    for tok in ORDER:
        if isinstance(tok, str):
            extract(int(tok[1]))
        else:
            round_(*tok)

    # ---------------- flags: raw per-partition sums, host adds -----------
    nc.sync.dma_start(out=flags_d[:, :], in_=BAD)


_NC_CACHE = None


def _build():
    global _NC_CACHE
    if _NC_CACHE is not None:
        return _NC_CACHE
    nc = bacc.Bacc(
        "TRN2", target_bir_lowering=False, debug=False, num_devices=N_CORES
    )
    x_d = nc.dram_tensor("tensor", [ROWS, E], F32, kind="ExternalInput").ap()
    out_d = nc.dram_tensor("out", [ROWS, E], I8, kind="ExternalOutput").ap()
    flags_d = nc.dram_tensor("flags", [P, 1], F32, kind="ExternalOutput").ap()
    from contextlib import ExitStack

    with tile.TileContext(nc) as tc, ExitStack() as ctx:
        _build_body(tc, x_d, out_d, flags_d, ctx)
    nc.compile()
    _NC_CACHE = nc
    return nc


_LAST_RESULTS = None


def kernel(tensor, boost_tensor, boost_percent):
    global _LAST_RESULTS
    tensor = np.ascontiguousarray(np.asarray(tensor, dtype=np.float32))
    boost_tensor = np.asarray(boost_tensor, dtype=np.float32)
    bp = np.float32(np.asarray(boost_percent, dtype=np.float32).reshape(-1)[0])

    # device path assumes boost_tensor == 0 (this module's forward contract);
    # exotic nonzero boosts take the exact host path
    if boost_tensor.any():
        return _host_reference(tensor, boost_tensor, float(bp))

    # c = relu(global max) * boost_percent, in fp32 exactly as the reference
    gmax = np.float32(max(np.float32(0.0), tensor.max()))
    cval = np.float32(gmax * bp)

    nc = _build()
    in_maps = []
    for c in range(N_CORES):
        sl = slice(c * ROWS, (c + 1) * ROWS)
        in_maps.append({"tensor": tensor[sl]})
    trace = bool(int(os.environ.get("KW_TRACE", "0")))
    res = run_bass_kernel_spmd(
        nc, in_maps, core_ids=list(range(N_CORES)), trace=trace
    )
    _LAST_RESULTS = res

    nbad = sum(float(np.asarray(r["flags"]).sum()) for r in res.results)
    if nbad > 0:
        return _host_reference(tensor, boost_tensor, float(bp))

    mask = np.concatenate(
        [np.asarray(r["out"]) for r in res.results], axis=0
    ) > 0
    out = mask.astype(np.float32)
    bo = np.where(mask, np.float32(0.0), cval).astype(np.float32)
    return out, bo


def _host_reference(tensor, boost_tensor, bp):
    x = tensor.astype(np.float32)
    b = np.broadcast_to(boost_tensor.astype(np.float32), x.shape)
    max_val = max(0.0, float(x.max()))
    boost = (b + np.float32(max_val * bp)).astype(np.float32)
    boosted = (np.where(x > 0, x, np.float32(0)) + boost).astype(np.float32)
    kth = np.partition(boosted, E - K, axis=1)[:, E - K]
    mask = boosted > kth[:, None]
    need = K - mask.sum(1)
    tie = (boosted == kth[:, None]) & ~mask
    csum = np.cumsum(tie, axis=1)
    mask |= tie & (csum <= need[:, None])
    out = (mask & (x > 0)).astype(np.float32)
    if out.sum() == 0:
        out = mask.astype(np.float32)
    bo = np.where(mask, np.float32(0), boost).astype(np.float32)
    return out, bo
